# revision 12
# baseline (speedup 1.0000x reference)
"""GCN layer (out = 0.1*h + 0.9*segment_sum(h[src], dst)) on 8 trn2 NeuronCores.

Sharding: dst-node-parallel. Core c owns dst rows [6250c, 6250(c+1)).
Edges are routed to the core owning their dst. Per core, edges are grouped
by 128-row dst tile and gathered from HBM (full h replicated per core, bf16)
with dma_gather (int16 indices -> two src chunks), then aggregated into PSUM
with one-hot selection matmuls: psum[d, f] += sum_e [dstl[e]==d] * h[src[e], f].
The residual is folded in as an extra "self" matmul with (1/9)*I, and the
final 0.9 scale is applied on PSUM evacuation (Activation engine).

bf16 datapath: gathered features, selection matrices, and the self rows are
bf16 (halves gather bytes, 4x faster PE matmuls vs fp32); accumulation stays
fp32 in PSUM, output is fp32.

Self-contained: hardcodes all shapes; builds + compiles the Bass kernel at
call time (layout group counts depend on the edge distribution).
"""
import numpy as np

from concourse import bacc, mybir
from concourse.tile import TileContext
from concourse.bass_utils import run_bass_kernel_spmd

N = 50000
D = 128
M = 8
RPC = 6250        # dst rows per core
TILE = 128
TPC = 49          # tiles per core (6272 rows, last 22 discarded)
NPAD = 50048      # h padded rows (>= 7*6250 + 6272)
CHUNK = 32768     # src chunk boundary (int16 index limit)
ST = 7            # tiles per supertile
NST = TPC // ST
ALPHA = 0.1
SENT = 512.0      # dstl sentinel (never equals iota 0..127; exact in bf16)
SCAP = 16         # max selection-matrix columns per S-build/matmul group
SCRATCH = 32768   # SWDGE descriptor carveout bytes/partition
GMAX = 2048       # max indices per dma_gather call (SCRATCH//16)

BF16 = mybir.dt.bfloat16
NP_BF16 = mybir.dt.np(BF16)
GW = 256          # gather slot width in bf16 elems (512B slots)

LAST_RESULT = None  # BassKernelResults of the most recent run (for test.py)


def _balance(src, dst):
    """Balanced node -> (core, row) assignment: deal nodes (heaviest first)
    in blocks of M to the M cores, greedily equalizing per-(tile, chunk)
    cell counts across cores. Returns (assign_core[n], assign_row[n],
    nodes_by_core: list of (node_ids, rows))."""
    d0 = np.bincount(dst[src < CHUNK], minlength=N)
    d1 = np.bincount(dst[src >= CHUNK], minlength=N)
    order = np.argsort(-(d0 + d1), kind="stable")
    assign_core = np.empty(N, dtype=np.int64)
    assign_row = np.empty(N, dtype=np.int64)
    nblocks = N // M  # 6250 blocks of 8 nodes; block b -> tile b//128, pos b%128
    cur0 = np.zeros(M, dtype=np.int64)
    cur1 = np.zeros(M, dtype=np.int64)
    for b in range(nblocks):
        t, p = b // TILE, b % TILE
        if p == 0:
            cur0[:] = 0
            cur1[:] = 0
        nodes = order[b * M:(b + 1) * M]
        nodes = nodes[np.argsort(-d0[nodes], kind="stable")]
        cores = np.argsort(cur0 * 4096 + cur1, kind="stable")
        assign_core[nodes] = cores
        assign_row[nodes] = t * TILE + p
        cur0[cores] += d0[nodes]
        cur1[cores] += d1[nodes]
    nodes_by_core = []
    for c in range(M):
        ids = np.nonzero(assign_core == c)[0]
        nodes_by_core.append((ids, assign_row[ids]))
    return assign_core, assign_row, nodes_by_core


def _prep(src, dst):
    E = src.shape[0]
    assign_core, assign_row, nodes_by_core = _balance(src, dst)
    core = assign_core[dst]
    row = assign_row[dst]
    tile_in_core = row // TILE
    chunk = (src >= CHUNK).astype(np.int64)
    ncell = M * TPC * 2
    cell = (core * TPC + tile_in_core) * 2 + chunk
    counts = np.bincount(cell, minlength=ncell).reshape(M, TPC, 2)
    # 16-aligned per-(tile, chunk) segment sizes (max over cores, SPMD-uniform)
    n16 = ((counts.max(axis=0) + 15) // 16) * 16          # [TPC, 2]

    order = np.argsort(cell, kind="stable")
    cell_sorted = cell[order]
    starts = np.zeros(ncell + 1, dtype=np.int64)
    np.cumsum(counts.reshape(-1), out=starts[1:])
    rank_sorted = np.arange(E, dtype=np.int64) - starts[cell_sorted]
    rank = np.empty(E, dtype=np.int64)
    rank[order] = rank_sorted

    # ---- call/slot/column/instance layout (core-independent) ----
    slot_base = np.zeros((TPC, 2), dtype=np.int64)   # slot base within call
    call_slots = np.zeros((NST, 2), dtype=np.int64)  # slots per (st, k) call
    call_col_base = np.zeros((NST, 2), dtype=np.int64)  # col base within st
    ncols_st = np.zeros(NST, dtype=np.int64)
    # per (t, k): range of call-relative columns [c_lo, c_hi], instance base
    c_lo = np.zeros((TPC, 2), dtype=np.int64)
    c_hi = np.zeros((TPC, 2), dtype=np.int64)
    inst_base = np.zeros((TPC, 2), dtype=np.int64)
    n_inst = 0
    for st in range(NST):
        st_cols = 0
        for k in range(2):
            call_col_base[st, k] = st_cols
            s = 0
            for t in range(st * ST, st * ST + ST):
                slot_base[t, k] = s
                if n16[t, k] > 0:
                    c_lo[t, k] = s // TILE
                    c_hi[t, k] = (s + n16[t, k] - 1) // TILE
                    inst_base[t, k] = n_inst
                    n_inst += c_hi[t, k] - c_lo[t, k] + 1
                else:
                    c_lo[t, k], c_hi[t, k] = 0, -1
                    inst_base[t, k] = n_inst
                s += n16[t, k]
            ns = -(-s // TILE) * TILE          # round call slots to 128
            call_slots[st, k] = ns
            st_cols += ns // TILE
        ncols_st[st] = st_cols

    idx_call_base = np.zeros((NST, 2), dtype=np.int64)
    b = [0, 0]
    for st in range(NST):
        for k in range(2):
            idx_call_base[st, k] = b[k]
            b[k] += call_slots[st, k]

    # per-edge positions
    t_g = tile_in_core
    e_slot = slot_base[t_g, chunk] + rank              # slot within call
    e_col_rel = e_slot // TILE                          # call-relative column
    e_part = e_slot % TILE
    e_inst = inst_base[t_g, chunk] + (e_col_rel - c_lo[t_g, chunk])

    per_core = []
    for c in range(M):
        m = core == c
        sc = src[m]
        ch, tg = chunk[m], t_g[m]

        flat_idx = [np.zeros(b[k], dtype=np.int16) for k in range(2)]
        for k in range(2):
            mk = ch == k
            pos = idx_call_base[tg[mk] // ST, k] + e_slot[m][mk]
            flat_idx[k][pos] = (sc[mk] - k * CHUNK).astype(np.int16)

        def wrap(flat, k):
            outs = []
            for st in range(NST):
                a = int(idx_call_base[st, k])
                n = int(call_slots[st, k])
                if n == 0:
                    continue
                blk = flat[a:a + n].reshape(n // 16, 16).T
                outs.append(np.tile(blk, (8, 1)))
            if not outs:
                return np.zeros((128, 1), np.int16)
            return np.ascontiguousarray(np.concatenate(outs, axis=1))

        idx0 = wrap(flat_idx[0], 0)
        idx1 = wrap(flat_idx[1], 1)

        dstl = np.full((TILE, max(n_inst, 1)), SENT, dtype=NP_BF16)
        dstl[e_part[m], e_inst[m]] = (row[m] - tg * TILE).astype(NP_BF16)

        per_core.append((idx0, idx1, np.ascontiguousarray(dstl)))

    layout = dict(nodes_by_core=nodes_by_core,
                  n16=n16, slot_base=slot_base, call_slots=call_slots,
                  call_col_base=call_col_base, ncols_st=ncols_st,
                  idx_call_base=idx_call_base, c_lo=c_lo, c_hi=c_hi,
                  inst_base=inst_base, n_inst=n_inst)
    return per_core, layout


def _build(layout, i0_cols, i1_cols, iters=1, mode="full"):
    call_slots = layout["call_slots"]
    call_col_base = layout["call_col_base"]
    ncols_st = layout["ncols_st"]
    idx_call_base = layout["idx_call_base"]
    c_lo, c_hi = layout["c_lo"], layout["c_hi"]
    inst_base = layout["inst_base"]
    ncols_total = int(max(layout["n_inst"], 1))
    max_ncols_st = int(ncols_st.max())
    max_ncols_tk = max(
        int(c_hi[t, k] - c_lo[t, k] + 1)
        for t in range(TPC) for k in range(2))

    nq = {"full": 4, "full2q": 2, "cheap_s": 4, "no_pe": 4}.get(mode, 1)
    nc = bacc.Bacc(None, target_bir_lowering=False,
                   dynamic_dma_scratch_size=SCRATCH,
                   num_swdge_queues=nq)
    qctr = [0]
    h_pad = nc.dram_tensor("h_pad", [NPAD, GW], BF16, kind="ExternalInput")
    # h rows for each core-local dst row, pre-transposed on host:
    # h_self_t[p, t*128 + d] = h[node(t*128 + p), d]
    h_self_t = nc.dram_tensor("h_self_t", [TILE, TPC * TILE], BF16,
                              kind="ExternalInput")
    iota_in = nc.dram_tensor("iota", [TILE, TILE], BF16, kind="ExternalInput")
    iotar_in = nc.dram_tensor("iotar", [TILE, TILE * SCAP], BF16,
                              kind="ExternalInput")
    selfsel_in = nc.dram_tensor("selfsel", [TILE, TILE], BF16,
                                kind="ExternalInput")
    idx0_in = nc.dram_tensor("idx0", [128, i0_cols], mybir.dt.int16,
                             kind="ExternalInput")
    idx1_in = nc.dram_tensor("idx1", [128, i1_cols], mybir.dt.int16,
                             kind="ExternalInput")
    dstl_in = nc.dram_tensor("dstl", [TILE, ncols_total], BF16,
                             kind="ExternalInput")
    out = nc.dram_tensor("out", [TPC * TILE, D], mybir.dt.float32,
                         kind="ExternalOutput")

    with TileContext(nc) as tc:
        with (
            tc.tile_pool(name="const", bufs=1) as cpool,
            tc.tile_pool(name="gbuf", bufs=2) as gpool,
            tc.tile_pool(name="idx", bufs=2) as ipool,
            tc.tile_pool(name="dstl", bufs=2) as dpool,
            tc.tile_pool(name="sel", bufs=2) as spool,
            tc.tile_pool(name="io", bufs=4) as iopool,
            tc.tile_pool(name="psum", bufs=4, space="PSUM") as ppool,
        ):
            iota_t = cpool.tile([TILE, TILE], BF16, tag="iota")
            nc.sync.dma_start(out=iota_t[:], in_=iota_in[:, :])
            iotar_t = cpool.tile([TILE, TILE * SCAP], BF16, tag="iotar")
            nc.sync.dma_start(out=iotar_t[:], in_=iotar_in[:, :])
            selfsel_t = cpool.tile([TILE, TILE], BF16, tag="selfsel")
            nc.sync.dma_start(out=selfsel_t[:], in_=selfsel_in[:, :])
            hs_all = cpool.tile([TILE, TPC * TILE], BF16, tag="hs")
            nc.sync.dma_start(out=hs_all[:], in_=h_self_t[:, :])
            dummy = None
            if mode == "copysb":
                dummy = cpool.tile([TILE, 16 * GW], BF16, tag="dummy")
                nc.sync.dma_start(
                    out=dummy[:], in_=h_pad[0:16, :].rearrange(
                        "(o c) d -> o (c d)", o=1).to_broadcast(
                        [TILE, 16 * GW]))

            if iters > 1:
                loop_cm = tc.For_i(
                    0, iters, 1,
                    hint_engines=(mybir.EngineType.Pool,
                                  mybir.EngineType.PE,
                                  mybir.EngineType.DVE,
                                  mybir.EngineType.SP,
                                  mybir.EngineType.Activation))
                loop_cm.__enter__()

            inst_col0 = 0
            for st in range(NST):
                n_inst_st = sum(
                    int(c_hi[t, k] - c_lo[t, k] + 1)
                    for t in range(st * ST, st * ST + ST) for k in range(2)
                    if c_hi[t, k] >= c_lo[t, k])
                gbuf = gpool.tile([TILE, max_ncols_st * GW], BF16, tag="gbuf")
                dstl_t = dpool.tile([TILE, max_ncols_st + ST * 2], BF16,
                                    tag="dstl")
                nc.sync.dma_start(
                    out=dstl_t[:, :n_inst_st],
                    in_=dstl_in[:, inst_col0:inst_col0 + n_inst_st],
                )
                for k, idx_in in ((0, idx0_in), (1, idx1_in)):
                    nslots = int(call_slots[st, k])
                    if nslots == 0:
                        continue
                    icols = nslots // 16
                    ibase = int(idx_call_base[st, k]) // 16
                    idx_t = ipool.tile([128, icols], mybir.dt.int16,
                                       tag=f"idx{k}")
                    nc.sync.dma_start(out=idx_t[:],
                                      in_=idx_in[:, ibase:ibase + icols])
                    a = int(call_col_base[st, k])
                    src_ap = h_pad[:CHUNK, :] if k == 0 else h_pad[CHUNK:, :]
                    # dma_gather is limited to <=GMAX indices per call
                    # (SWDGE descriptor carveout limit).
                    for ci_call, p0 in enumerate(range(0, nslots, GMAX)):
                        ps = min(GMAX, nslots - p0)
                        pcols = -(-ps // TILE)
                        ac = a + p0 // TILE
                        gview = gbuf[:, ac * GW:(ac + pcols) * GW].rearrange(
                            "p (c d) -> p c d", d=GW)
                        if mode == "copy":
                            nc.sync.dma_start(
                                out=gview,
                                in_=h_pad[0:pcols * TILE, :].rearrange(
                                    "(c p) d -> p c d", p=TILE))
                        elif mode == "copy2e":
                            eng = nc.sync if ci_call % 2 == 0 else nc.scalar
                            eng.dma_start(
                                out=gview,
                                in_=h_pad[0:pcols * TILE, :].rearrange(
                                    "(c p) d -> p c d", p=TILE))
                        elif mode == "copy1k":
                            pc2 = max(pcols // 2, 1)
                            gv2 = gbuf[:, ac * GW:(ac + 2 * pc2) * GW
                                       ].rearrange("p (c d) -> p c d",
                                                   d=2 * GW)
                            nc.sync.dma_start(
                                out=gv2,
                                in_=h_pad[0:pc2 * TILE * 2, :].rearrange(
                                    "(c p two) d -> p c (two d)",
                                    p=TILE, two=2))
                        elif mode == "copysb":
                            nc.sync.dma_start(
                                out=gview,
                                in_=dummy[:, :pcols * GW].rearrange(
                                    "p (c d) -> p c d", d=GW))
                        else:
                            nc.gpsimd.dma_gather(
                                gview, src_ap,
                                idx_t[:, p0 // 16:(p0 + ps) // 16],
                                ps, ps, GW, queue_num=qctr[0] % nq)
                            qctr[0] += 1

                for t in range(st * ST, st * ST + ST):
                    if mode == "gather_only":
                        osb = iopool.tile([TILE, D], mybir.dt.float32,
                                          tag="osb")
                        nc.vector.tensor_scalar_mul(
                            osb[:], gbuf[:, :D], 1.0)
                        nc.sync.dma_start(
                            out=out[t * TILE:(t + 1) * TILE, :], in_=osb[:])
                        continue
                    psum = ppool.tile([TILE, TILE], mybir.dt.float32, tag="ps")
                    first = True
                    for k in range(2):
                        if c_hi[t, k] < c_lo[t, k]:
                            continue
                        ncols_tk = int(c_hi[t, k] - c_lo[t, k] + 1)
                        ji0 = int(inst_base[t, k]) - inst_col0
                        for g0 in range(0, ncols_tk, SCAP):
                            gn = min(SCAP, ncols_tk - g0)
                            S = spool.tile([TILE, SCAP * TILE], BF16,
                                           tag="S")
                            # d-major S: S[p, d*gn + c] = (dstl[p, c] == d);
                            # all operands 2-byte packed-last for DVE 2x mode
                            nc.vector.tensor_tensor(
                                out=S[:, :gn * TILE].rearrange(
                                    "p (d c) -> p d c", c=gn),
                                in0=dstl_t[:, ji0 + g0:ji0 + g0 + gn
                                           ].rearrange(
                                    "p (o c) -> p o c", o=1).to_broadcast(
                                    [TILE, TILE, gn]),
                                in1=iotar_t[:, :].rearrange(
                                    "p (d c) -> p d c", c=SCAP)[:, :, :gn],
                                op=mybir.AluOpType.is_equal,
                            )
                            if mode == "no_pe":
                                continue
                            S_r = S[:, :gn * TILE].rearrange(
                                "p (d c) -> p c d", c=gn)
                            for ci in range(gn):
                                jr = int(call_col_base[st, k] + c_lo[t, k]
                                         ) + g0 + ci
                                nc.tensor.matmul(
                                    out=psum[:],
                                    lhsT=S_r[:, ci:ci + 1, :],
                                    rhs=gbuf[:, jr * GW:jr * GW + D],
                                    start=first,
                                    stop=False,
                                )
                                first = False
                    nc.tensor.matmul(
                        out=psum[:], lhsT=selfsel_t[:],
                        rhs=hs_all[:, t * TILE:(t + 1) * TILE],
                        start=first, stop=True)
                    osb = iopool.tile([TILE, D], mybir.dt.float32, tag="osb")
                    if mode == "no_pe":
                        nc.scalar.activation(
                            osb[:], hs_all[:, t * TILE:(t + 1) * TILE],
                            mybir.ActivationFunctionType.Copy,
                            scale=1.0 - ALPHA)
                    else:
                        nc.scalar.activation(
                            osb[:], psum[:], mybir.ActivationFunctionType.Copy,
                            scale=1.0 - ALPHA)
                    nc.sync.dma_start(
                        out=out[t * TILE:(t + 1) * TILE, :], in_=osb[:])
                inst_col0 += n_inst_st
            if iters > 1:
                loop_cm.__exit__(None, None, None)
    nc.compile()
    return nc


def build_and_inputs(h, src, dst):
    """Returns (nc, in_maps) for the 8-core SPMD kernel."""
    h = np.ascontiguousarray(np.asarray(h, dtype=np.float32))
    src = np.asarray(src).astype(np.int64)
    dst = np.asarray(dst).astype(np.int64)

    per_core, layout = _prep(src, dst)
    h_bf = h.astype(NP_BF16)
    h_pad = np.zeros((NPAD, GW), NP_BF16)
    h_pad[:N, :D] = h_bf
    iota = np.broadcast_to(np.arange(TILE, dtype=np.float32), (TILE, TILE))
    iota = np.ascontiguousarray(iota.astype(NP_BF16))
    iotar = np.repeat(np.arange(TILE, dtype=np.float32), SCAP)
    iotar = np.broadcast_to(iotar, (TILE, TILE * SCAP))
    iotar = np.ascontiguousarray(iotar.astype(NP_BF16))
    selfsel = np.ascontiguousarray(
        (np.eye(TILE, dtype=np.float32) * (ALPHA / (1.0 - ALPHA))
         ).astype(NP_BF16))

    i0_cols = max(pc[0].shape[1] for pc in per_core)
    i1_cols = max(pc[1].shape[1] for pc in per_core)
    nc = _build(layout, i0_cols, i1_cols)

    in_maps = []
    for c in range(M):
        idx0, idx1, dstl = per_core[c]
        ids, rows = layout["nodes_by_core"][c]
        h_self = np.zeros((TPC * TILE, D), NP_BF16)
        h_self[rows] = h_bf[ids]
        h_self_t = np.ascontiguousarray(
            h_self.reshape(TPC, TILE, D).transpose(1, 0, 2).reshape(
                TILE, TPC * D))
        in_maps.append({
            "h_pad": h_pad,
            "h_self_t": h_self_t,
            "iota": iota,
            "iotar": iotar,
            "selfsel": selfsel,
            "idx0": idx0,
            "idx1": idx1,
            "dstl": dstl,
        })
    return nc, in_maps, layout


def kernel(h, src, dst, **_):
    global LAST_RESULT
    import os
    # NTFF tracing needs an axon hook that is absent in this environment;
    # make sure a stray BASS_TRACE can't break execution.
    os.environ["BASS_NEVER_TRACE"] = "1"
    nc, in_maps, layout = build_and_inputs(h, src, dst)
    res = run_bass_kernel_spmd(nc, in_maps, core_ids=list(range(M)))
    LAST_RESULT = res
    out = np.empty((N, D), np.float32)
    for c in range(M):
        ids, rows = layout["nodes_by_core"][c]
        out[ids] = res.results[c]["out"][rows]
    return out


# revision 17
# speedup vs baseline: 1.0606x; 1.0606x over previous
"""GCN layer (out = 0.1*h + 0.9*segment_sum(h[src], dst)) on 8 trn2 NeuronCores.

Sharding: dst-node-parallel. Core c owns dst rows [6250c, 6250(c+1)).
Edges are routed to the core owning their dst. Per core, edges are grouped
by 128-row dst tile and gathered from HBM (full h replicated per core, bf16)
with dma_gather (int16 indices -> two src chunks), then aggregated into PSUM
with one-hot selection matmuls: psum[d, f] += sum_e [dstl[e]==d] * h[src[e], f].
The residual is folded in as an extra "self" matmul with (1/9)*I, and the
final 0.9 scale is applied on PSUM evacuation (Activation engine).

bf16 datapath: gathered features, selection matrices, and the self rows are
bf16 (halves gather bytes, 4x faster PE matmuls vs fp32); accumulation stays
fp32 in PSUM, output is fp32.

Self-contained: hardcodes all shapes; builds + compiles the Bass kernel at
call time (layout group counts depend on the edge distribution).
"""
import numpy as np

from concourse import bacc, mybir
from concourse.tile import TileContext
from concourse.bass_utils import run_bass_kernel_spmd

N = 50000
D = 128
M = 8
RPC = 6250        # dst rows per core
TILE = 128
TPC = 49          # tiles per core (6272 rows, last 22 discarded)
NPAD = 50048      # h padded rows (>= 7*6250 + 6272)
CHUNK = 32768     # src chunk boundary (int16 index limit)
ST = 7            # tiles per supertile
NST = TPC // ST
ALPHA = 0.1
SENT = 512.0      # dstl sentinel (never equals iota 0..127; exact in bf16)
SCAP = 12         # max selection-matrix columns per S-build/matmul group
SCRATCH = 32768   # SWDGE descriptor carveout bytes/partition
GMAX = 2048       # max indices per dma_gather call (SCRATCH//16)

BF16 = mybir.dt.bfloat16
NP_BF16 = mybir.dt.np(BF16)
GW = 256          # gather slot width in bf16 elems (512B slots)

LAST_RESULT = None  # BassKernelResults of the most recent run (for test.py)


def _balance(src, dst):
    """Balanced node -> (core, row) assignment: deal nodes (heaviest first)
    in blocks of M to the M cores, greedily equalizing per-(tile, chunk)
    cell counts across cores. Returns (assign_core[n], assign_row[n],
    nodes_by_core: list of (node_ids, rows))."""
    d0 = np.bincount(dst[src < CHUNK], minlength=N)
    d1 = np.bincount(dst[src >= CHUNK], minlength=N)
    order = np.argsort(-(d0 + d1), kind="stable")
    assign_core = np.empty(N, dtype=np.int64)
    assign_row = np.empty(N, dtype=np.int64)
    nblocks = N // M  # 6250 blocks of 8 nodes; block b -> tile b//128, pos b%128
    cur0 = np.zeros(M, dtype=np.int64)
    cur1 = np.zeros(M, dtype=np.int64)
    for b in range(nblocks):
        t, p = b // TILE, b % TILE
        if p == 0:
            cur0[:] = 0
            cur1[:] = 0
        nodes = order[b * M:(b + 1) * M]
        nodes = nodes[np.argsort(-d0[nodes], kind="stable")]
        cores = np.argsort(cur0 * 4096 + cur1, kind="stable")
        assign_core[nodes] = cores
        assign_row[nodes] = t * TILE + p
        cur0[cores] += d0[nodes]
        cur1[cores] += d1[nodes]
    nodes_by_core = []
    for c in range(M):
        ids = np.nonzero(assign_core == c)[0]
        nodes_by_core.append((ids, assign_row[ids]))
    return assign_core, assign_row, nodes_by_core


def _prep(src, dst):
    E = src.shape[0]
    assign_core, assign_row, nodes_by_core = _balance(src, dst)
    core = assign_core[dst]
    row = assign_row[dst]
    tile_in_core = row // TILE
    chunk = (src >= CHUNK).astype(np.int64)
    ncell = M * TPC * 2
    cell = (core * TPC + tile_in_core) * 2 + chunk
    counts = np.bincount(cell, minlength=ncell).reshape(M, TPC, 2)
    # 16-aligned per-(tile, chunk) segment sizes (max over cores, SPMD-uniform)
    n16 = ((counts.max(axis=0) + 15) // 16) * 16          # [TPC, 2]

    order = np.argsort(cell, kind="stable")
    cell_sorted = cell[order]
    starts = np.zeros(ncell + 1, dtype=np.int64)
    np.cumsum(counts.reshape(-1), out=starts[1:])
    rank_sorted = np.arange(E, dtype=np.int64) - starts[cell_sorted]
    rank = np.empty(E, dtype=np.int64)
    rank[order] = rank_sorted

    # ---- call/slot/column/instance layout (core-independent) ----
    slot_base = np.zeros((TPC, 2), dtype=np.int64)   # slot base within call
    call_slots = np.zeros((NST, 2), dtype=np.int64)  # slots per (st, k) call
    call_col_base = np.zeros((NST, 2), dtype=np.int64)  # col base within st
    ncols_st = np.zeros(NST, dtype=np.int64)
    # per (t, k): range of call-relative columns [c_lo, c_hi], instance base
    c_lo = np.zeros((TPC, 2), dtype=np.int64)
    c_hi = np.zeros((TPC, 2), dtype=np.int64)
    inst_base = np.zeros((TPC, 2), dtype=np.int64)
    n_inst = 0
    for st in range(NST):
        st_cols = 0
        for k in range(2):
            call_col_base[st, k] = st_cols
            s = 0
            for t in range(st * ST, st * ST + ST):
                slot_base[t, k] = s
                if n16[t, k] > 0:
                    c_lo[t, k] = s // TILE
                    c_hi[t, k] = (s + n16[t, k] - 1) // TILE
                    inst_base[t, k] = n_inst
                    n_inst += c_hi[t, k] - c_lo[t, k] + 1
                else:
                    c_lo[t, k], c_hi[t, k] = 0, -1
                    inst_base[t, k] = n_inst
                s += n16[t, k]
            ns = -(-s // TILE) * TILE          # round call slots to 128
            call_slots[st, k] = ns
            st_cols += ns // TILE
        ncols_st[st] = st_cols

    idx_call_base = np.zeros((NST, 2), dtype=np.int64)
    b = [0, 0]
    for st in range(NST):
        for k in range(2):
            idx_call_base[st, k] = b[k]
            b[k] += call_slots[st, k]

    # per-edge positions
    t_g = tile_in_core
    e_slot = slot_base[t_g, chunk] + rank              # slot within call
    e_col_rel = e_slot // TILE                          # call-relative column
    e_part = e_slot % TILE
    e_inst = inst_base[t_g, chunk] + (e_col_rel - c_lo[t_g, chunk])

    per_core = []
    for c in range(M):
        m = core == c
        sc = src[m]
        ch, tg = chunk[m], t_g[m]

        flat_idx = [np.zeros(b[k], dtype=np.int16) for k in range(2)]
        for k in range(2):
            mk = ch == k
            pos = idx_call_base[tg[mk] // ST, k] + e_slot[m][mk]
            flat_idx[k][pos] = (sc[mk] - k * CHUNK).astype(np.int16)

        def wrap(flat, k):
            outs = []
            for st in range(NST):
                a = int(idx_call_base[st, k])
                n = int(call_slots[st, k])
                if n == 0:
                    continue
                blk = flat[a:a + n].reshape(n // 16, 16).T
                outs.append(np.tile(blk, (8, 1)))
            if not outs:
                return np.zeros((128, 1), np.int16)
            return np.ascontiguousarray(np.concatenate(outs, axis=1))

        idx0 = wrap(flat_idx[0], 0)
        idx1 = wrap(flat_idx[1], 1)

        dstl = np.full((TILE, max(n_inst, 1)), SENT, dtype=NP_BF16)
        dstl[e_part[m], e_inst[m]] = (row[m] - tg * TILE).astype(NP_BF16)

        per_core.append((idx0, idx1, np.ascontiguousarray(dstl)))

    layout = dict(nodes_by_core=nodes_by_core,
                  n16=n16, slot_base=slot_base, call_slots=call_slots,
                  call_col_base=call_col_base, ncols_st=ncols_st,
                  idx_call_base=idx_call_base, c_lo=c_lo, c_hi=c_hi,
                  inst_base=inst_base, n_inst=n_inst)
    return per_core, layout


def _build(layout, i0_cols, i1_cols, iters=1, mode="full"):
    call_slots = layout["call_slots"]
    call_col_base = layout["call_col_base"]
    ncols_st = layout["ncols_st"]
    idx_call_base = layout["idx_call_base"]
    c_lo, c_hi = layout["c_lo"], layout["c_hi"]
    inst_base = layout["inst_base"]
    ncols_total = int(max(layout["n_inst"], 1))
    max_ncols_st = int(ncols_st.max())
    max_ncols_tk = max(
        int(c_hi[t, k] - c_lo[t, k] + 1)
        for t in range(TPC) for k in range(2))

    nq = 4  # SWDGE queues (ucode max); gathers rotate across them so
    # descriptor generation pipelines with ring draining
    nc = bacc.Bacc(None, target_bir_lowering=False,
                   dynamic_dma_scratch_size=SCRATCH,
                   num_swdge_queues=nq)
    qctr = [0]
    h_pad = nc.dram_tensor("h_pad", [NPAD, GW], BF16, kind="ExternalInput")
    # h rows for each core-local dst row, pre-transposed on host:
    # h_self_t[p, t*128 + d] = h[node(t*128 + p), d]
    h_self_t = nc.dram_tensor("h_self_t", [TILE, TPC * TILE], BF16,
                              kind="ExternalInput")
    iota_in = nc.dram_tensor("iota", [TILE, TILE], BF16, kind="ExternalInput")
    iotar_in = nc.dram_tensor("iotar", [TILE, TILE * SCAP], BF16,
                              kind="ExternalInput")
    selfsel_in = nc.dram_tensor("selfsel", [TILE, TILE], BF16,
                                kind="ExternalInput")
    idx0_in = nc.dram_tensor("idx0", [128, i0_cols], mybir.dt.int16,
                             kind="ExternalInput")
    idx1_in = nc.dram_tensor("idx1", [128, i1_cols], mybir.dt.int16,
                             kind="ExternalInput")
    dstl_in = nc.dram_tensor("dstl", [TILE, ncols_total], BF16,
                             kind="ExternalInput")
    out = nc.dram_tensor("out", [TPC * TILE, D], BF16,
                         kind="ExternalOutput")

    with TileContext(nc) as tc:
        with (
            tc.tile_pool(name="const", bufs=1) as cpool,
            tc.tile_pool(name="gbuf", bufs=2) as gpool,
            tc.tile_pool(name="idx", bufs=2) as ipool,
            tc.tile_pool(name="dstl", bufs=2) as dpool,
            tc.tile_pool(name="sel", bufs=2) as spool,
            tc.tile_pool(name="io", bufs=2) as iopool,
            tc.tile_pool(name="psum", bufs=4, space="PSUM") as ppool,
        ):
            iota_t = cpool.tile([TILE, TILE], BF16, tag="iota")
            nc.sync.dma_start(out=iota_t[:], in_=iota_in[:, :])
            iotar_t = cpool.tile([TILE, TILE * SCAP], BF16, tag="iotar")
            nc.sync.dma_start(out=iotar_t[:], in_=iotar_in[:, :])
            selfsel_t = cpool.tile([TILE, TILE], BF16, tag="selfsel")
            nc.sync.dma_start(out=selfsel_t[:], in_=selfsel_in[:, :])
            hs_all = cpool.tile([TILE, TPC * TILE], BF16, tag="hs")
            nc.sync.dma_start(out=hs_all[:], in_=h_self_t[:, :])

            if iters > 1:
                loop_cm = tc.For_i(
                    0, iters, 1,
                    hint_engines=(mybir.EngineType.Pool,
                                  mybir.EngineType.PE,
                                  mybir.EngineType.DVE,
                                  mybir.EngineType.SP,
                                  mybir.EngineType.Activation))
                loop_cm.__enter__()

            inst_col0 = 0
            for st in range(NST):
                n_inst_st = sum(
                    int(c_hi[t, k] - c_lo[t, k] + 1)
                    for t in range(st * ST, st * ST + ST) for k in range(2)
                    if c_hi[t, k] >= c_lo[t, k])
                gbuf = gpool.tile([TILE, max_ncols_st * GW], BF16, tag="gbuf")
                dstl_t = dpool.tile([TILE, max_ncols_st + ST * 2], BF16,
                                    tag="dstl")
                nc.sync.dma_start(
                    out=dstl_t[:, :n_inst_st],
                    in_=dstl_in[:, inst_col0:inst_col0 + n_inst_st],
                )
                for k, idx_in in ((0, idx0_in), (1, idx1_in)):
                    nslots = int(call_slots[st, k])
                    if nslots == 0:
                        continue
                    icols = nslots // 16
                    ibase = int(idx_call_base[st, k]) // 16
                    idx_t = ipool.tile([128, icols], mybir.dt.int16,
                                       tag=f"idx{k}")
                    nc.sync.dma_start(out=idx_t[:],
                                      in_=idx_in[:, ibase:ibase + icols])
                    a = int(call_col_base[st, k])
                    src_ap = h_pad[:CHUNK, :] if k == 0 else h_pad[CHUNK:, :]
                    # dma_gather is limited to <=GMAX indices per call
                    # (SWDGE descriptor carveout limit).
                    for p0 in range(0, nslots, GMAX):
                        ps = min(GMAX, nslots - p0)
                        pcols = -(-ps // TILE)
                        ac = a + p0 // TILE
                        gview = gbuf[:, ac * GW:(ac + pcols) * GW].rearrange(
                            "p (c d) -> p c d", d=GW)
                        nc.gpsimd.dma_gather(
                            gview, src_ap,
                            idx_t[:, p0 // 16:(p0 + ps) // 16],
                            ps, ps, GW, queue_num=qctr[0] % nq)
                        qctr[0] += 1

                for t in range(st * ST, st * ST + ST):
                    psum = ppool.tile([TILE, TILE], mybir.dt.float32, tag="ps")
                    first = True
                    for k in range(2):
                        if c_hi[t, k] < c_lo[t, k]:
                            continue
                        ncols_tk = int(c_hi[t, k] - c_lo[t, k] + 1)
                        ji0 = int(inst_base[t, k]) - inst_col0
                        for g0 in range(0, ncols_tk, SCAP):
                            gn = min(SCAP, ncols_tk - g0)
                            S = spool.tile([TILE, SCAP * TILE], BF16,
                                           tag="S")
                            # d-major S: S[p, d*gn + c] = (dstl[p, c] == d);
                            # all operands 2-byte packed-last for DVE 2x mode
                            nc.vector.tensor_tensor(
                                out=S[:, :gn * TILE].rearrange(
                                    "p (d c) -> p d c", c=gn),
                                in0=dstl_t[:, ji0 + g0:ji0 + g0 + gn
                                           ].rearrange(
                                    "p (o c) -> p o c", o=1).to_broadcast(
                                    [TILE, TILE, gn]),
                                in1=iotar_t[:, :].rearrange(
                                    "p (d c) -> p d c", c=SCAP)[:, :, :gn],
                                op=mybir.AluOpType.is_equal,
                            )
                            S_r = S[:, :gn * TILE].rearrange(
                                "p (d c) -> p c d", c=gn)
                            for ci in range(gn):
                                jr = int(call_col_base[st, k] + c_lo[t, k]
                                         ) + g0 + ci
                                nc.tensor.matmul(
                                    out=psum[:],
                                    lhsT=S_r[:, ci:ci + 1, :],
                                    rhs=gbuf[:, jr * GW:jr * GW + D],
                                    start=first,
                                    stop=False,
                                )
                                first = False
                    nc.tensor.matmul(
                        out=psum[:], lhsT=selfsel_t[:],
                        rhs=hs_all[:, t * TILE:(t + 1) * TILE],
                        start=first, stop=True)
                    osb = iopool.tile([TILE, D], BF16, tag="osb")
                    nc.scalar.activation(
                        osb[:], psum[:], mybir.ActivationFunctionType.Copy,
                        scale=1.0 - ALPHA)
                    nc.sync.dma_start(
                        out=out[t * TILE:(t + 1) * TILE, :], in_=osb[:])
                inst_col0 += n_inst_st
            if iters > 1:
                loop_cm.__exit__(None, None, None)
    nc.compile()
    return nc


def build_and_inputs(h, src, dst):
    """Returns (nc, in_maps) for the 8-core SPMD kernel."""
    h = np.ascontiguousarray(np.asarray(h, dtype=np.float32))
    src = np.asarray(src).astype(np.int64)
    dst = np.asarray(dst).astype(np.int64)

    per_core, layout = _prep(src, dst)
    h_bf = h.astype(NP_BF16)
    h_pad = np.zeros((NPAD, GW), NP_BF16)
    h_pad[:N, :D] = h_bf
    iota = np.broadcast_to(np.arange(TILE, dtype=np.float32), (TILE, TILE))
    iota = np.ascontiguousarray(iota.astype(NP_BF16))
    iotar = np.repeat(np.arange(TILE, dtype=np.float32), SCAP)
    iotar = np.broadcast_to(iotar, (TILE, TILE * SCAP))
    iotar = np.ascontiguousarray(iotar.astype(NP_BF16))
    selfsel = np.ascontiguousarray(
        (np.eye(TILE, dtype=np.float32) * (ALPHA / (1.0 - ALPHA))
         ).astype(NP_BF16))

    i0_cols = max(pc[0].shape[1] for pc in per_core)
    i1_cols = max(pc[1].shape[1] for pc in per_core)
    nc = _build(layout, i0_cols, i1_cols)

    in_maps = []
    for c in range(M):
        idx0, idx1, dstl = per_core[c]
        ids, rows = layout["nodes_by_core"][c]
        h_self = np.zeros((TPC * TILE, D), NP_BF16)
        h_self[rows] = h_bf[ids]
        h_self_t = np.ascontiguousarray(
            h_self.reshape(TPC, TILE, D).transpose(1, 0, 2).reshape(
                TILE, TPC * D))
        in_maps.append({
            "h_pad": h_pad,
            "h_self_t": h_self_t,
            "iota": iota,
            "iotar": iotar,
            "selfsel": selfsel,
            "idx0": idx0,
            "idx1": idx1,
            "dstl": dstl,
        })
    return nc, in_maps, layout


def kernel(h, src, dst, **_):
    global LAST_RESULT
    import os
    # NTFF tracing needs an axon hook that is absent in this environment;
    # make sure a stray BASS_TRACE can't break execution.
    os.environ["BASS_NEVER_TRACE"] = "1"
    nc, in_maps, layout = build_and_inputs(h, src, dst)
    res = run_bass_kernel_spmd(nc, in_maps, core_ids=list(range(M)))
    LAST_RESULT = res
    out = np.empty((N, D), np.float32)
    for c in range(M):
        ids, rows = layout["nodes_by_core"][c]
        out[ids] = res.results[c]["out"][rows].astype(np.float32)
    return out


# revision 20
# speedup vs baseline: 1.3147x; 1.2395x over previous
"""GCN layer (out = 0.1*h + 0.9*segment_sum(h[src], dst)) on 8 trn2 NeuronCores.

Sharding: dst-node-parallel. Core c owns dst rows [6250c, 6250(c+1)).
Edges are routed to the core owning their dst. Per core, edges are grouped
by 128-row dst tile and gathered from HBM (full h replicated per core, bf16)
with dma_gather (int16 indices -> two src chunks), then aggregated into PSUM
with one-hot selection matmuls: psum[d, f] += sum_e [dstl[e]==d] * h[src[e], f].
The residual is folded in as an extra "self" matmul with (1/9)*I, and the
final 0.9 scale is applied on PSUM evacuation (Activation engine).

bf16 datapath: gathered features, selection matrices, and the self rows are
bf16 (halves gather bytes, 4x faster PE matmuls vs fp32); accumulation stays
fp32 in PSUM, output is fp32.

Self-contained: hardcodes all shapes; builds + compiles the Bass kernel at
call time (layout group counts depend on the edge distribution).
"""
import numpy as np

from concourse import bacc, mybir
from concourse.tile import TileContext
from concourse.bass_utils import run_bass_kernel_spmd

N = 50000
D = 128
M = 8
RPC = 6250        # dst rows per core
TILE = 128
TPC = 49          # tiles per core (6272 rows, last 22 discarded)
NPAD = 50048      # h padded rows (>= 7*6250 + 6272)
CHUNK = 32768     # src chunk boundary (int16 index limit)
ST = 7            # tiles per supertile
NST = TPC // ST
ALPHA = 0.1
SENT = 512.0      # dstl sentinel (never equals iota 0..127; exact in bf16)
SCAP = 12         # max selection-matrix columns per S-build/matmul group
SCRATCH = 32768   # SWDGE descriptor carveout bytes/partition
GMAX = 2048       # max indices per dma_gather call (SCRATCH//16)

import os as _os
BF16 = mybir.dt.bfloat16
NP_BF16 = mybir.dt.np(BF16)
GW = int(_os.environ.get("KGW", "128"))  # gather slot width in bf16 elems

LAST_RESULT = None  # BassKernelResults of the most recent run (for test.py)


def _balance(src, dst):
    """Balanced node -> (core, row) assignment: deal nodes (heaviest first)
    in blocks of M to the M cores, greedily equalizing per-(tile, chunk)
    cell counts across cores. Returns (assign_core[n], assign_row[n],
    nodes_by_core: list of (node_ids, rows))."""
    d0 = np.bincount(dst[src < CHUNK], minlength=N)
    d1 = np.bincount(dst[src >= CHUNK], minlength=N)
    order = np.argsort(-(d0 + d1), kind="stable")
    assign_core = np.empty(N, dtype=np.int64)
    assign_row = np.empty(N, dtype=np.int64)
    nblocks = N // M  # 6250 blocks of 8 nodes; block b -> tile b//128, pos b%128
    cur0 = np.zeros(M, dtype=np.int64)
    cur1 = np.zeros(M, dtype=np.int64)
    for b in range(nblocks):
        t, p = b // TILE, b % TILE
        if p == 0:
            cur0[:] = 0
            cur1[:] = 0
        nodes = order[b * M:(b + 1) * M]
        nodes = nodes[np.argsort(-d0[nodes], kind="stable")]
        cores = np.argsort(cur0 * 4096 + cur1, kind="stable")
        assign_core[nodes] = cores
        assign_row[nodes] = t * TILE + p
        cur0[cores] += d0[nodes]
        cur1[cores] += d1[nodes]
    nodes_by_core = []
    for c in range(M):
        ids = np.nonzero(assign_core == c)[0]
        nodes_by_core.append((ids, assign_row[ids]))
    return assign_core, assign_row, nodes_by_core


def _prep(src, dst):
    E = src.shape[0]
    assign_core, assign_row, nodes_by_core = _balance(src, dst)
    core = assign_core[dst]
    row = assign_row[dst]
    tile_in_core = row // TILE
    chunk = (src >= CHUNK).astype(np.int64)
    ncell = M * TPC * 2
    cell = (core * TPC + tile_in_core) * 2 + chunk
    counts = np.bincount(cell, minlength=ncell).reshape(M, TPC, 2)
    # 16-aligned per-(tile, chunk) segment sizes (max over cores, SPMD-uniform)
    n16 = ((counts.max(axis=0) + 15) // 16) * 16          # [TPC, 2]

    order = np.argsort(cell, kind="stable")
    cell_sorted = cell[order]
    starts = np.zeros(ncell + 1, dtype=np.int64)
    np.cumsum(counts.reshape(-1), out=starts[1:])
    rank_sorted = np.arange(E, dtype=np.int64) - starts[cell_sorted]
    rank = np.empty(E, dtype=np.int64)
    rank[order] = rank_sorted

    # ---- call/slot/column/instance layout (core-independent) ----
    slot_base = np.zeros((TPC, 2), dtype=np.int64)   # slot base within call
    call_slots = np.zeros((NST, 2), dtype=np.int64)  # slots per (st, k) call
    call_col_base = np.zeros((NST, 2), dtype=np.int64)  # col base within st
    ncols_st = np.zeros(NST, dtype=np.int64)
    # per (t, k): range of call-relative columns [c_lo, c_hi], instance base
    c_lo = np.zeros((TPC, 2), dtype=np.int64)
    c_hi = np.zeros((TPC, 2), dtype=np.int64)
    inst_base = np.zeros((TPC, 2), dtype=np.int64)
    n_inst = 0
    for st in range(NST):
        st_cols = 0
        for k in range(2):
            call_col_base[st, k] = st_cols
            s = 0
            for t in range(st * ST, st * ST + ST):
                slot_base[t, k] = s
                if n16[t, k] > 0:
                    c_lo[t, k] = s // TILE
                    c_hi[t, k] = (s + n16[t, k] - 1) // TILE
                    inst_base[t, k] = n_inst
                    n_inst += c_hi[t, k] - c_lo[t, k] + 1
                else:
                    c_lo[t, k], c_hi[t, k] = 0, -1
                    inst_base[t, k] = n_inst
                s += n16[t, k]
            ns = -(-s // TILE) * TILE          # round call slots to 128
            call_slots[st, k] = ns
            st_cols += ns // TILE
        ncols_st[st] = st_cols

    idx_call_base = np.zeros((NST, 2), dtype=np.int64)
    b = [0, 0]
    for st in range(NST):
        for k in range(2):
            idx_call_base[st, k] = b[k]
            b[k] += call_slots[st, k]

    # per-edge positions
    t_g = tile_in_core
    e_slot = slot_base[t_g, chunk] + rank              # slot within call
    e_col_rel = e_slot // TILE                          # call-relative column
    e_part = e_slot % TILE
    e_inst = inst_base[t_g, chunk] + (e_col_rel - c_lo[t_g, chunk])

    per_core = []
    for c in range(M):
        m = core == c
        sc = src[m]
        ch, tg = chunk[m], t_g[m]

        flat_idx = [np.zeros(b[k], dtype=np.int16) for k in range(2)]
        for k in range(2):
            mk = ch == k
            pos = idx_call_base[tg[mk] // ST, k] + e_slot[m][mk]
            flat_idx[k][pos] = (sc[mk] - k * CHUNK).astype(np.int16)

        def wrap(flat, k):
            outs = []
            for st in range(NST):
                a = int(idx_call_base[st, k])
                n = int(call_slots[st, k])
                if n == 0:
                    continue
                blk = flat[a:a + n].reshape(n // 16, 16).T
                outs.append(np.tile(blk, (8, 1)))
            if not outs:
                return np.zeros((128, 1), np.int16)
            return np.ascontiguousarray(np.concatenate(outs, axis=1))

        idx0 = wrap(flat_idx[0], 0)
        idx1 = wrap(flat_idx[1], 1)

        dstl = np.full((TILE, max(n_inst, 1)), SENT, dtype=NP_BF16)
        dstl[e_part[m], e_inst[m]] = (row[m] - tg * TILE).astype(NP_BF16)

        per_core.append((idx0, idx1, np.ascontiguousarray(dstl)))

    layout = dict(nodes_by_core=nodes_by_core,
                  n16=n16, slot_base=slot_base, call_slots=call_slots,
                  call_col_base=call_col_base, ncols_st=ncols_st,
                  idx_call_base=idx_call_base, c_lo=c_lo, c_hi=c_hi,
                  inst_base=inst_base, n_inst=n_inst)
    return per_core, layout


def _build(layout, i0_cols, i1_cols, iters=1, mode="full"):
    call_slots = layout["call_slots"]
    call_col_base = layout["call_col_base"]
    ncols_st = layout["ncols_st"]
    idx_call_base = layout["idx_call_base"]
    c_lo, c_hi = layout["c_lo"], layout["c_hi"]
    inst_base = layout["inst_base"]
    ncols_total = int(max(layout["n_inst"], 1))
    max_ncols_st = int(ncols_st.max())
    max_ncols_tk = max(
        int(c_hi[t, k] - c_lo[t, k] + 1)
        for t in range(TPC) for k in range(2))

    nq = 4  # SWDGE queues (ucode max); gathers rotate across them so
    # descriptor generation pipelines with ring draining
    nc = bacc.Bacc(None, target_bir_lowering=False,
                   dynamic_dma_scratch_size=SCRATCH,
                   num_swdge_queues=nq)
    qctr = [0]
    h_pad = nc.dram_tensor("h_pad", [NPAD, GW], BF16, kind="ExternalInput")
    # h rows for each core-local dst row, pre-transposed on host:
    # h_self_t[p, t*128 + d] = h[node(t*128 + p), d]
    h_self_t = nc.dram_tensor("h_self_t", [TILE, TPC * TILE], BF16,
                              kind="ExternalInput")
    iota_in = nc.dram_tensor("iota", [TILE, TILE], BF16, kind="ExternalInput")
    iotar_in = nc.dram_tensor("iotar", [TILE, TILE * SCAP], BF16,
                              kind="ExternalInput")
    selfsel_in = nc.dram_tensor("selfsel", [TILE, TILE], BF16,
                                kind="ExternalInput")
    idx0_in = nc.dram_tensor("idx0", [128, i0_cols], mybir.dt.int16,
                             kind="ExternalInput")
    idx1_in = nc.dram_tensor("idx1", [128, i1_cols], mybir.dt.int16,
                             kind="ExternalInput")
    dstl_in = nc.dram_tensor("dstl", [TILE, ncols_total], BF16,
                             kind="ExternalInput")
    out = nc.dram_tensor("out", [TPC * TILE, D], BF16,
                         kind="ExternalOutput")

    with TileContext(nc) as tc:
        with (
            tc.tile_pool(name="const", bufs=1) as cpool,
            tc.tile_pool(name="gbuf", bufs=4) as gpool,
            tc.tile_pool(name="idx", bufs=4) as ipool,
            tc.tile_pool(name="dstl", bufs=3) as dpool,
            tc.tile_pool(name="sel", bufs=2) as spool,
            tc.tile_pool(name="io", bufs=2) as iopool,
            tc.tile_pool(name="psum", bufs=4, space="PSUM") as ppool,
        ):
            iota_t = cpool.tile([TILE, TILE], BF16, tag="iota")
            nc.sync.dma_start(out=iota_t[:], in_=iota_in[:, :])
            iotar_t = cpool.tile([TILE, TILE * SCAP], BF16, tag="iotar")
            nc.sync.dma_start(out=iotar_t[:], in_=iotar_in[:, :])
            selfsel_t = cpool.tile([TILE, TILE], BF16, tag="selfsel")
            nc.sync.dma_start(out=selfsel_t[:], in_=selfsel_in[:, :])
            hs_all = cpool.tile([TILE, TPC * TILE], BF16, tag="hs")
            nc.sync.dma_start(out=hs_all[:], in_=h_self_t[:, :])

            if iters > 1:
                loop_cm = tc.For_i(
                    0, iters, 1,
                    hint_engines=(mybir.EngineType.Pool,
                                  mybir.EngineType.PE,
                                  mybir.EngineType.DVE,
                                  mybir.EngineType.SP,
                                  mybir.EngineType.Activation))
                loop_cm.__enter__()

            inst_col0 = 0
            for st in range(NST):
                n_inst_st = sum(
                    int(c_hi[t, k] - c_lo[t, k] + 1)
                    for t in range(st * ST, st * ST + ST) for k in range(2)
                    if c_hi[t, k] >= c_lo[t, k])
                gbuf = gpool.tile([TILE, max_ncols_st * GW], BF16, tag="gbuf")
                dstl_t = dpool.tile([TILE, max_ncols_st + ST * 2], BF16,
                                    tag="dstl")
                nc.sync.dma_start(
                    out=dstl_t[:, :n_inst_st],
                    in_=dstl_in[:, inst_col0:inst_col0 + n_inst_st],
                )
                for k, idx_in in ((0, idx0_in), (1, idx1_in)):
                    nslots = int(call_slots[st, k])
                    if nslots == 0:
                        continue
                    icols = nslots // 16
                    ibase = int(idx_call_base[st, k]) // 16
                    idx_t = ipool.tile([128, icols], mybir.dt.int16,
                                       tag=f"idx{k}")
                    nc.sync.dma_start(out=idx_t[:],
                                      in_=idx_in[:, ibase:ibase + icols])
                    a = int(call_col_base[st, k])
                    src_ap = h_pad[:CHUNK, :] if k == 0 else h_pad[CHUNK:, :]
                    # dma_gather is limited to <=GMAX indices per call
                    # (SWDGE descriptor carveout limit).
                    for p0 in range(0, nslots, GMAX):
                        ps = min(GMAX, nslots - p0)
                        pcols = -(-ps // TILE)
                        ac = a + p0 // TILE
                        gview = gbuf[:, ac * GW:(ac + pcols) * GW].rearrange(
                            "p (c d) -> p c d", d=GW)
                        nc.gpsimd.dma_gather(
                            gview, src_ap,
                            idx_t[:, p0 // 16:(p0 + ps) // 16],
                            ps, ps, GW, queue_num=qctr[0] % nq)
                        qctr[0] += 1

                for t in range(st * ST, st * ST + ST):
                    psum = ppool.tile([TILE, TILE], mybir.dt.float32, tag="ps")
                    first = True
                    for k in range(2):
                        if c_hi[t, k] < c_lo[t, k]:
                            continue
                        ncols_tk = int(c_hi[t, k] - c_lo[t, k] + 1)
                        ji0 = int(inst_base[t, k]) - inst_col0
                        for g0 in range(0, ncols_tk, SCAP):
                            gn = min(SCAP, ncols_tk - g0)
                            S = spool.tile([TILE, SCAP * TILE], BF16,
                                           tag="S")
                            # d-major S: S[p, d*gn + c] = (dstl[p, c] == d);
                            # all operands 2-byte packed-last for DVE 2x mode
                            nc.vector.tensor_tensor(
                                out=S[:, :gn * TILE].rearrange(
                                    "p (d c) -> p d c", c=gn),
                                in0=dstl_t[:, ji0 + g0:ji0 + g0 + gn
                                           ].rearrange(
                                    "p (o c) -> p o c", o=1).to_broadcast(
                                    [TILE, TILE, gn]),
                                in1=iotar_t[:, :].rearrange(
                                    "p (d c) -> p d c", c=SCAP)[:, :, :gn],
                                op=mybir.AluOpType.is_equal,
                            )
                            S_r = S[:, :gn * TILE].rearrange(
                                "p (d c) -> p c d", c=gn)
                            for ci in range(gn):
                                jr = int(call_col_base[st, k] + c_lo[t, k]
                                         ) + g0 + ci
                                nc.tensor.matmul(
                                    out=psum[:],
                                    lhsT=S_r[:, ci:ci + 1, :],
                                    rhs=gbuf[:, jr * GW:jr * GW + D],
                                    start=first,
                                    stop=False,
                                )
                                first = False
                    nc.tensor.matmul(
                        out=psum[:], lhsT=selfsel_t[:],
                        rhs=hs_all[:, t * TILE:(t + 1) * TILE],
                        start=first, stop=True)
                    osb = iopool.tile([TILE, D], BF16, tag="osb")
                    nc.scalar.activation(
                        osb[:], psum[:], mybir.ActivationFunctionType.Copy,
                        scale=1.0 - ALPHA)
                    nc.sync.dma_start(
                        out=out[t * TILE:(t + 1) * TILE, :], in_=osb[:])
                inst_col0 += n_inst_st
            if iters > 1:
                loop_cm.__exit__(None, None, None)
    nc.compile()
    return nc


def build_and_inputs(h, src, dst):
    """Returns (nc, in_maps) for the 8-core SPMD kernel."""
    h = np.ascontiguousarray(np.asarray(h, dtype=np.float32))
    src = np.asarray(src).astype(np.int64)
    dst = np.asarray(dst).astype(np.int64)

    per_core, layout = _prep(src, dst)
    h_bf = h.astype(NP_BF16)
    h_pad = np.zeros((NPAD, GW), NP_BF16)
    h_pad[:N, :D] = h_bf
    iota = np.broadcast_to(np.arange(TILE, dtype=np.float32), (TILE, TILE))
    iota = np.ascontiguousarray(iota.astype(NP_BF16))
    iotar = np.repeat(np.arange(TILE, dtype=np.float32), SCAP)
    iotar = np.broadcast_to(iotar, (TILE, TILE * SCAP))
    iotar = np.ascontiguousarray(iotar.astype(NP_BF16))
    selfsel = np.ascontiguousarray(
        (np.eye(TILE, dtype=np.float32) * (ALPHA / (1.0 - ALPHA))
         ).astype(NP_BF16))

    i0_cols = max(pc[0].shape[1] for pc in per_core)
    i1_cols = max(pc[1].shape[1] for pc in per_core)
    nc = _build(layout, i0_cols, i1_cols)

    in_maps = []
    for c in range(M):
        idx0, idx1, dstl = per_core[c]
        ids, rows = layout["nodes_by_core"][c]
        h_self = np.zeros((TPC * TILE, D), NP_BF16)
        h_self[rows] = h_bf[ids]
        h_self_t = np.ascontiguousarray(
            h_self.reshape(TPC, TILE, D).transpose(1, 0, 2).reshape(
                TILE, TPC * D))
        in_maps.append({
            "h_pad": h_pad,
            "h_self_t": h_self_t,
            "iota": iota,
            "iotar": iotar,
            "selfsel": selfsel,
            "idx0": idx0,
            "idx1": idx1,
            "dstl": dstl,
        })
    return nc, in_maps, layout


def kernel(h, src, dst, **_):
    global LAST_RESULT
    import os
    # NTFF tracing needs an axon hook that is absent in this environment;
    # make sure a stray BASS_TRACE can't break execution.
    os.environ["BASS_NEVER_TRACE"] = "1"
    nc, in_maps, layout = build_and_inputs(h, src, dst)
    res = run_bass_kernel_spmd(nc, in_maps, core_ids=list(range(M)))
    LAST_RESULT = res
    out = np.empty((N, D), np.float32)
    for c in range(M):
        ids, rows = layout["nodes_by_core"][c]
        out[ids] = res.results[c]["out"][rows].astype(np.float32)
    return out


# revision 21
# speedup vs baseline: 1.3545x; 1.0303x over previous
"""GCN layer (out = 0.1*h + 0.9*segment_sum(h[src], dst)) on 8 trn2 NeuronCores.

Sharding: dst-node-parallel. Core c owns dst rows [6250c, 6250(c+1)).
Edges are routed to the core owning their dst. Per core, edges are grouped
by 128-row dst tile and gathered from HBM (full h replicated per core, bf16)
with dma_gather (int16 indices -> two src chunks), then aggregated into PSUM
with one-hot selection matmuls: psum[d, f] += sum_e [dstl[e]==d] * h[src[e], f].
The residual is folded in as an extra "self" matmul with (1/9)*I, and the
final 0.9 scale is applied on PSUM evacuation (Activation engine).

bf16 datapath: gathered features, selection matrices, and the self rows are
bf16 (halves gather bytes, 4x faster PE matmuls vs fp32); accumulation stays
fp32 in PSUM, output is fp32.

Self-contained: hardcodes all shapes; builds + compiles the Bass kernel at
call time (layout group counts depend on the edge distribution).
"""
import numpy as np

from concourse import bacc, mybir
from concourse.tile import TileContext
from concourse.bass_utils import run_bass_kernel_spmd

N = 50000
D = 128
M = 8
RPC = 6250        # dst rows per core
TILE = 128
TPC = 49          # tiles per core (6272 rows, last 22 discarded)
NPAD = 50048      # h padded rows (>= 7*6250 + 6272)
CHUNK = 32768     # src chunk boundary (int16 index limit)
ST = 7            # tiles per supertile
NST = TPC // ST
ALPHA = 0.1
SENT = 512.0      # dstl sentinel (never equals iota 0..127; exact in bf16)
SCAP = 12         # max selection-matrix columns per S-build/matmul group
SCRATCH = 32768   # SWDGE descriptor carveout bytes/partition
GMAX = 2048       # max indices per dma_gather call (SCRATCH//16)

import os as _os
BF16 = mybir.dt.bfloat16
NP_BF16 = mybir.dt.np(BF16)
GW = int(_os.environ.get("KGW", "128"))  # gather slot width in bf16 elems
SP = _os.environ.get("KSP", "1") == "1"   # dma_gather single_packet
GMAXE = int(_os.environ.get("KGMAX", str(GMAX)))  # effective gather call size

LAST_RESULT = None  # BassKernelResults of the most recent run (for test.py)


def _balance(src, dst):
    """Balanced node -> (core, row) assignment: deal nodes (heaviest first)
    in blocks of M to the M cores, greedily equalizing per-(tile, chunk)
    cell counts across cores. Returns (assign_core[n], assign_row[n],
    nodes_by_core: list of (node_ids, rows))."""
    d0 = np.bincount(dst[src < CHUNK], minlength=N)
    d1 = np.bincount(dst[src >= CHUNK], minlength=N)
    order = np.argsort(-(d0 + d1), kind="stable")
    assign_core = np.empty(N, dtype=np.int64)
    assign_row = np.empty(N, dtype=np.int64)
    nblocks = N // M  # 6250 blocks of 8 nodes; block b -> tile b//128, pos b%128
    cur0 = np.zeros(M, dtype=np.int64)
    cur1 = np.zeros(M, dtype=np.int64)
    for b in range(nblocks):
        t, p = b // TILE, b % TILE
        if p == 0:
            cur0[:] = 0
            cur1[:] = 0
        nodes = order[b * M:(b + 1) * M]
        nodes = nodes[np.argsort(-d0[nodes], kind="stable")]
        cores = np.argsort(cur0 * 4096 + cur1, kind="stable")
        assign_core[nodes] = cores
        assign_row[nodes] = t * TILE + p
        cur0[cores] += d0[nodes]
        cur1[cores] += d1[nodes]
    nodes_by_core = []
    for c in range(M):
        ids = np.nonzero(assign_core == c)[0]
        nodes_by_core.append((ids, assign_row[ids]))
    return assign_core, assign_row, nodes_by_core


def _prep(src, dst):
    E = src.shape[0]
    assign_core, assign_row, nodes_by_core = _balance(src, dst)
    core = assign_core[dst]
    row = assign_row[dst]
    tile_in_core = row // TILE
    chunk = (src >= CHUNK).astype(np.int64)
    ncell = M * TPC * 2
    cell = (core * TPC + tile_in_core) * 2 + chunk
    counts = np.bincount(cell, minlength=ncell).reshape(M, TPC, 2)
    # 16-aligned per-(tile, chunk) segment sizes (max over cores, SPMD-uniform)
    n16 = ((counts.max(axis=0) + 15) // 16) * 16          # [TPC, 2]

    order = np.argsort(cell, kind="stable")
    cell_sorted = cell[order]
    starts = np.zeros(ncell + 1, dtype=np.int64)
    np.cumsum(counts.reshape(-1), out=starts[1:])
    rank_sorted = np.arange(E, dtype=np.int64) - starts[cell_sorted]
    rank = np.empty(E, dtype=np.int64)
    rank[order] = rank_sorted

    # ---- call/slot/column/instance layout (core-independent) ----
    slot_base = np.zeros((TPC, 2), dtype=np.int64)   # slot base within call
    call_slots = np.zeros((NST, 2), dtype=np.int64)  # slots per (st, k) call
    call_col_base = np.zeros((NST, 2), dtype=np.int64)  # col base within st
    ncols_st = np.zeros(NST, dtype=np.int64)
    # per (t, k): range of call-relative columns [c_lo, c_hi], instance base
    c_lo = np.zeros((TPC, 2), dtype=np.int64)
    c_hi = np.zeros((TPC, 2), dtype=np.int64)
    inst_base = np.zeros((TPC, 2), dtype=np.int64)
    n_inst = 0
    for st in range(NST):
        st_cols = 0
        for k in range(2):
            call_col_base[st, k] = st_cols
            s = 0
            for t in range(st * ST, st * ST + ST):
                slot_base[t, k] = s
                if n16[t, k] > 0:
                    c_lo[t, k] = s // TILE
                    c_hi[t, k] = (s + n16[t, k] - 1) // TILE
                    inst_base[t, k] = n_inst
                    n_inst += c_hi[t, k] - c_lo[t, k] + 1
                else:
                    c_lo[t, k], c_hi[t, k] = 0, -1
                    inst_base[t, k] = n_inst
                s += n16[t, k]
            ns = -(-s // TILE) * TILE          # round call slots to 128
            call_slots[st, k] = ns
            st_cols += ns // TILE
        ncols_st[st] = st_cols

    idx_call_base = np.zeros((NST, 2), dtype=np.int64)
    b = [0, 0]
    for st in range(NST):
        for k in range(2):
            idx_call_base[st, k] = b[k]
            b[k] += call_slots[st, k]

    # per-edge positions
    t_g = tile_in_core
    e_slot = slot_base[t_g, chunk] + rank              # slot within call
    e_col_rel = e_slot // TILE                          # call-relative column
    e_part = e_slot % TILE
    e_inst = inst_base[t_g, chunk] + (e_col_rel - c_lo[t_g, chunk])

    per_core = []
    for c in range(M):
        m = core == c
        sc = src[m]
        ch, tg = chunk[m], t_g[m]

        flat_idx = [np.zeros(b[k], dtype=np.int16) for k in range(2)]
        for k in range(2):
            mk = ch == k
            pos = idx_call_base[tg[mk] // ST, k] + e_slot[m][mk]
            flat_idx[k][pos] = (sc[mk] - k * CHUNK).astype(np.int16)

        def wrap(flat, k):
            outs = []
            for st in range(NST):
                a = int(idx_call_base[st, k])
                n = int(call_slots[st, k])
                if n == 0:
                    continue
                blk = flat[a:a + n].reshape(n // 16, 16).T
                outs.append(np.tile(blk, (8, 1)))
            if not outs:
                return np.zeros((128, 1), np.int16)
            return np.ascontiguousarray(np.concatenate(outs, axis=1))

        idx0 = wrap(flat_idx[0], 0)
        idx1 = wrap(flat_idx[1], 1)

        dstl = np.full((TILE, max(n_inst, 1)), SENT, dtype=NP_BF16)
        dstl[e_part[m], e_inst[m]] = (row[m] - tg * TILE).astype(NP_BF16)

        per_core.append((idx0, idx1, np.ascontiguousarray(dstl)))

    layout = dict(nodes_by_core=nodes_by_core,
                  n16=n16, slot_base=slot_base, call_slots=call_slots,
                  call_col_base=call_col_base, ncols_st=ncols_st,
                  idx_call_base=idx_call_base, c_lo=c_lo, c_hi=c_hi,
                  inst_base=inst_base, n_inst=n_inst)
    return per_core, layout


def _build(layout, i0_cols, i1_cols, iters=1, mode="full"):
    call_slots = layout["call_slots"]
    call_col_base = layout["call_col_base"]
    ncols_st = layout["ncols_st"]
    idx_call_base = layout["idx_call_base"]
    c_lo, c_hi = layout["c_lo"], layout["c_hi"]
    inst_base = layout["inst_base"]
    ncols_total = int(max(layout["n_inst"], 1))
    max_ncols_st = int(ncols_st.max())
    max_ncols_tk = max(
        int(c_hi[t, k] - c_lo[t, k] + 1)
        for t in range(TPC) for k in range(2))

    nq = 4  # SWDGE queues (ucode max); gathers rotate across them so
    # descriptor generation pipelines with ring draining
    nc = bacc.Bacc(None, target_bir_lowering=False,
                   dynamic_dma_scratch_size=SCRATCH,
                   num_swdge_queues=nq)
    qctr = [0]
    h_pad = nc.dram_tensor("h_pad", [NPAD, GW], BF16, kind="ExternalInput")
    # h rows for each core-local dst row, pre-transposed on host:
    # h_self_t[p, t*128 + d] = h[node(t*128 + p), d]
    h_self_t = nc.dram_tensor("h_self_t", [TILE, TPC * TILE], BF16,
                              kind="ExternalInput")
    iota_in = nc.dram_tensor("iota", [TILE, TILE], BF16, kind="ExternalInput")
    iotar_in = nc.dram_tensor("iotar", [TILE, TILE * SCAP], BF16,
                              kind="ExternalInput")
    selfsel_in = nc.dram_tensor("selfsel", [TILE, TILE], BF16,
                                kind="ExternalInput")
    idx0_in = nc.dram_tensor("idx0", [128, i0_cols], mybir.dt.int16,
                             kind="ExternalInput")
    idx1_in = nc.dram_tensor("idx1", [128, i1_cols], mybir.dt.int16,
                             kind="ExternalInput")
    dstl_in = nc.dram_tensor("dstl", [TILE, ncols_total], BF16,
                             kind="ExternalInput")
    out = nc.dram_tensor("out", [TPC * TILE, D], BF16,
                         kind="ExternalOutput")

    with TileContext(nc) as tc:
        with (
            tc.tile_pool(name="const", bufs=1) as cpool,
            tc.tile_pool(name="gbuf", bufs=4) as gpool,
            tc.tile_pool(name="idx", bufs=4) as ipool,
            tc.tile_pool(name="dstl", bufs=3) as dpool,
            tc.tile_pool(name="sel", bufs=2) as spool,
            tc.tile_pool(name="io", bufs=2) as iopool,
            tc.tile_pool(name="psum", bufs=4, space="PSUM") as ppool,
        ):
            iota_t = cpool.tile([TILE, TILE], BF16, tag="iota")
            nc.sync.dma_start(out=iota_t[:], in_=iota_in[:, :])
            iotar_t = cpool.tile([TILE, TILE * SCAP], BF16, tag="iotar")
            nc.sync.dma_start(out=iotar_t[:], in_=iotar_in[:, :])
            selfsel_t = cpool.tile([TILE, TILE], BF16, tag="selfsel")
            nc.sync.dma_start(out=selfsel_t[:], in_=selfsel_in[:, :])
            hs_all = cpool.tile([TILE, TPC * TILE], BF16, tag="hs")
            nc.sync.dma_start(out=hs_all[:], in_=h_self_t[:, :])

            if iters > 1:
                loop_cm = tc.For_i(
                    0, iters, 1,
                    hint_engines=(mybir.EngineType.Pool,
                                  mybir.EngineType.PE,
                                  mybir.EngineType.DVE,
                                  mybir.EngineType.SP,
                                  mybir.EngineType.Activation))
                loop_cm.__enter__()

            inst_col0 = 0
            for st in range(NST):
                n_inst_st = sum(
                    int(c_hi[t, k] - c_lo[t, k] + 1)
                    for t in range(st * ST, st * ST + ST) for k in range(2)
                    if c_hi[t, k] >= c_lo[t, k])
                gbuf = gpool.tile([TILE, max_ncols_st * GW], BF16, tag="gbuf")
                dstl_t = dpool.tile([TILE, max_ncols_st + ST * 2], BF16,
                                    tag="dstl")
                nc.sync.dma_start(
                    out=dstl_t[:, :n_inst_st],
                    in_=dstl_in[:, inst_col0:inst_col0 + n_inst_st],
                )
                for k, idx_in in ((0, idx0_in), (1, idx1_in)):
                    nslots = int(call_slots[st, k])
                    if nslots == 0:
                        continue
                    icols = nslots // 16
                    ibase = int(idx_call_base[st, k]) // 16
                    idx_t = ipool.tile([128, icols], mybir.dt.int16,
                                       tag=f"idx{k}")
                    nc.sync.dma_start(out=idx_t[:],
                                      in_=idx_in[:, ibase:ibase + icols])
                    a = int(call_col_base[st, k])
                    src_ap = h_pad[:CHUNK, :] if k == 0 else h_pad[CHUNK:, :]
                    # dma_gather is limited to <=GMAX indices per call
                    # (SWDGE descriptor carveout limit).
                    for p0 in range(0, nslots, GMAXE):
                        ps = min(GMAXE, nslots - p0)
                        pcols = -(-ps // TILE)
                        ac = a + p0 // TILE
                        gview = gbuf[:, ac * GW:(ac + pcols) * GW].rearrange(
                            "p (c d) -> p c d", d=GW)
                        nc.gpsimd.dma_gather(
                            gview, src_ap,
                            idx_t[:, p0 // 16:(p0 + ps) // 16],
                            ps, ps, GW, queue_num=qctr[0] % nq,
                            single_packet=SP)
                        qctr[0] += 1

                for t in range(st * ST, st * ST + ST):
                    psum = ppool.tile([TILE, TILE], mybir.dt.float32, tag="ps")
                    first = True
                    for k in range(2):
                        if c_hi[t, k] < c_lo[t, k]:
                            continue
                        ncols_tk = int(c_hi[t, k] - c_lo[t, k] + 1)
                        ji0 = int(inst_base[t, k]) - inst_col0
                        for g0 in range(0, ncols_tk, SCAP):
                            gn = min(SCAP, ncols_tk - g0)
                            S = spool.tile([TILE, SCAP * TILE], BF16,
                                           tag="S")
                            # d-major S: S[p, d*gn + c] = (dstl[p, c] == d);
                            # all operands 2-byte packed-last for DVE 2x mode
                            nc.vector.tensor_tensor(
                                out=S[:, :gn * TILE].rearrange(
                                    "p (d c) -> p d c", c=gn),
                                in0=dstl_t[:, ji0 + g0:ji0 + g0 + gn
                                           ].rearrange(
                                    "p (o c) -> p o c", o=1).to_broadcast(
                                    [TILE, TILE, gn]),
                                in1=iotar_t[:, :].rearrange(
                                    "p (d c) -> p d c", c=SCAP)[:, :, :gn],
                                op=mybir.AluOpType.is_equal,
                            )
                            S_r = S[:, :gn * TILE].rearrange(
                                "p (d c) -> p c d", c=gn)
                            for ci in range(gn):
                                jr = int(call_col_base[st, k] + c_lo[t, k]
                                         ) + g0 + ci
                                nc.tensor.matmul(
                                    out=psum[:],
                                    lhsT=S_r[:, ci:ci + 1, :],
                                    rhs=gbuf[:, jr * GW:jr * GW + D],
                                    start=first,
                                    stop=False,
                                )
                                first = False
                    nc.tensor.matmul(
                        out=psum[:], lhsT=selfsel_t[:],
                        rhs=hs_all[:, t * TILE:(t + 1) * TILE],
                        start=first, stop=True)
                    osb = iopool.tile([TILE, D], BF16, tag="osb")
                    nc.scalar.activation(
                        osb[:], psum[:], mybir.ActivationFunctionType.Copy,
                        scale=1.0 - ALPHA)
                    nc.sync.dma_start(
                        out=out[t * TILE:(t + 1) * TILE, :], in_=osb[:])
                inst_col0 += n_inst_st
            if iters > 1:
                loop_cm.__exit__(None, None, None)
    nc.compile()
    return nc


def build_and_inputs(h, src, dst):
    """Returns (nc, in_maps) for the 8-core SPMD kernel."""
    h = np.ascontiguousarray(np.asarray(h, dtype=np.float32))
    src = np.asarray(src).astype(np.int64)
    dst = np.asarray(dst).astype(np.int64)

    per_core, layout = _prep(src, dst)
    h_bf = h.astype(NP_BF16)
    h_pad = np.zeros((NPAD, GW), NP_BF16)
    h_pad[:N, :D] = h_bf
    iota = np.broadcast_to(np.arange(TILE, dtype=np.float32), (TILE, TILE))
    iota = np.ascontiguousarray(iota.astype(NP_BF16))
    iotar = np.repeat(np.arange(TILE, dtype=np.float32), SCAP)
    iotar = np.broadcast_to(iotar, (TILE, TILE * SCAP))
    iotar = np.ascontiguousarray(iotar.astype(NP_BF16))
    selfsel = np.ascontiguousarray(
        (np.eye(TILE, dtype=np.float32) * (ALPHA / (1.0 - ALPHA))
         ).astype(NP_BF16))

    i0_cols = max(pc[0].shape[1] for pc in per_core)
    i1_cols = max(pc[1].shape[1] for pc in per_core)
    nc = _build(layout, i0_cols, i1_cols)

    in_maps = []
    for c in range(M):
        idx0, idx1, dstl = per_core[c]
        ids, rows = layout["nodes_by_core"][c]
        h_self = np.zeros((TPC * TILE, D), NP_BF16)
        h_self[rows] = h_bf[ids]
        h_self_t = np.ascontiguousarray(
            h_self.reshape(TPC, TILE, D).transpose(1, 0, 2).reshape(
                TILE, TPC * D))
        in_maps.append({
            "h_pad": h_pad,
            "h_self_t": h_self_t,
            "iota": iota,
            "iotar": iotar,
            "selfsel": selfsel,
            "idx0": idx0,
            "idx1": idx1,
            "dstl": dstl,
        })
    return nc, in_maps, layout


def kernel(h, src, dst, **_):
    global LAST_RESULT
    import os
    # NTFF tracing needs an axon hook that is absent in this environment;
    # make sure a stray BASS_TRACE can't break execution.
    os.environ["BASS_NEVER_TRACE"] = "1"
    nc, in_maps, layout = build_and_inputs(h, src, dst)
    res = run_bass_kernel_spmd(nc, in_maps, core_ids=list(range(M)))
    LAST_RESULT = res
    out = np.empty((N, D), np.float32)
    for c in range(M):
        ids, rows = layout["nodes_by_core"][c]
        out[ids] = res.results[c]["out"][rows].astype(np.float32)
    return out


# revision 23
# speedup vs baseline: 1.5417x; 1.1382x over previous
"""GCN layer (out = 0.1*h + 0.9*segment_sum(h[src], dst)) on 8 trn2 NeuronCores.

Sharding: dst-node-parallel. Core c owns dst rows [6250c, 6250(c+1)).
Edges are routed to the core owning their dst. Per core, edges are grouped
by 128-row dst tile and gathered from HBM (full h replicated per core, bf16)
with dma_gather (int16 indices -> two src chunks), then aggregated into PSUM
with one-hot selection matmuls: psum[d, f] += sum_e [dstl[e]==d] * h[src[e], f].
The residual is folded in as an extra "self" matmul with (1/9)*I, and the
final 0.9 scale is applied on PSUM evacuation (Activation engine).

bf16 datapath: gathered features, selection matrices, and the self rows are
bf16 (halves gather bytes, 4x faster PE matmuls vs fp32); accumulation stays
fp32 in PSUM, output is fp32.

Self-contained: hardcodes all shapes; builds + compiles the Bass kernel at
call time (layout group counts depend on the edge distribution).
"""
import numpy as np

from concourse import bacc, mybir
from concourse.tile import TileContext
from concourse.bass_utils import run_bass_kernel_spmd

N = 50000
D = 128
M = 8
RPC = 6250        # dst rows per core
TILE = 128
TPC = 49          # tiles per core (6272 rows, last 22 discarded)
NPAD = 50048      # h padded rows (>= 7*6250 + 6272)
CHUNK = 32768     # src chunk boundary (int16 index limit)
ST = 7            # tiles per supertile
NST = TPC // ST
ALPHA = 0.1
SENT = 512.0      # dstl sentinel (never equals iota 0..127; exact in bf16)
SCAP = 12         # max selection-matrix columns per S-build/matmul group
SCRATCH = 32768   # SWDGE descriptor carveout bytes/partition
GMAX = 2048       # max indices per dma_gather call (SCRATCH//16)

import os as _os
BF16 = mybir.dt.bfloat16
NP_BF16 = mybir.dt.np(BF16)
GW = int(_os.environ.get("KGW", "128"))  # gather slot width in bf16 elems
SP = _os.environ.get("KSP", "1") == "1"   # dma_gather single_packet
GMAXE = int(_os.environ.get("KGMAX", str(GMAX)))  # effective gather call size

LAST_RESULT = None  # BassKernelResults of the most recent run (for test.py)


def _balance(src, dst):
    """Balanced node -> (core, row) assignment: deal nodes (heaviest first)
    in blocks of M to the M cores, greedily equalizing per-(tile, chunk)
    cell counts across cores. Returns (assign_core[n], assign_row[n],
    nodes_by_core: list of (node_ids, rows))."""
    d0 = np.bincount(dst[src < CHUNK], minlength=N)
    d1 = np.bincount(dst[src >= CHUNK], minlength=N)
    order = np.argsort(-(d0 + d1), kind="stable")
    assign_core = np.empty(N, dtype=np.int64)
    assign_row = np.empty(N, dtype=np.int64)
    nblocks = N // M  # 6250 blocks of 8 nodes; block b -> tile b//128, pos b%128
    cur0 = np.zeros(M, dtype=np.int64)
    cur1 = np.zeros(M, dtype=np.int64)
    for b in range(nblocks):
        t, p = b // TILE, b % TILE
        if p == 0:
            cur0[:] = 0
            cur1[:] = 0
        nodes = order[b * M:(b + 1) * M]
        nodes = nodes[np.argsort(-d0[nodes], kind="stable")]
        cores = np.argsort(cur0 * 4096 + cur1, kind="stable")
        assign_core[nodes] = cores
        assign_row[nodes] = t * TILE + p
        cur0[cores] += d0[nodes]
        cur1[cores] += d1[nodes]
    nodes_by_core = []
    for c in range(M):
        ids = np.nonzero(assign_core == c)[0]
        nodes_by_core.append((ids, assign_row[ids]))
    return assign_core, assign_row, nodes_by_core


def _prep(src, dst):
    E = src.shape[0]
    assign_core, assign_row, nodes_by_core = _balance(src, dst)
    core = assign_core[dst]
    row = assign_row[dst]
    tile_in_core = row // TILE
    chunk = (src >= CHUNK).astype(np.int64)
    ncell = M * TPC * 2
    cell = (core * TPC + tile_in_core) * 2 + chunk
    counts = np.bincount(cell, minlength=ncell).reshape(M, TPC, 2)
    # 16-aligned per-(tile, chunk) segment sizes (max over cores, SPMD-uniform)
    n16 = ((counts.max(axis=0) + 15) // 16) * 16          # [TPC, 2]

    order = np.argsort(cell, kind="stable")
    cell_sorted = cell[order]
    starts = np.zeros(ncell + 1, dtype=np.int64)
    np.cumsum(counts.reshape(-1), out=starts[1:])
    rank_sorted = np.arange(E, dtype=np.int64) - starts[cell_sorted]
    rank = np.empty(E, dtype=np.int64)
    rank[order] = rank_sorted

    # ---- call/slot/column/instance layout (core-independent) ----
    slot_base = np.zeros((TPC, 2), dtype=np.int64)   # slot base within call
    call_slots = np.zeros((NST, 2), dtype=np.int64)  # slots per (st, k) call
    call_col_base = np.zeros((NST, 2), dtype=np.int64)  # col base within st
    ncols_st = np.zeros(NST, dtype=np.int64)
    # per (t, k): range of call-relative columns [c_lo, c_hi], instance base
    c_lo = np.zeros((TPC, 2), dtype=np.int64)
    c_hi = np.zeros((TPC, 2), dtype=np.int64)
    inst_base = np.zeros((TPC, 2), dtype=np.int64)
    n_inst = 0
    for st in range(NST):
        st_cols = 0
        for k in range(2):
            call_col_base[st, k] = st_cols
            s = 0
            for t in range(st * ST, st * ST + ST):
                slot_base[t, k] = s
                if n16[t, k] > 0:
                    c_lo[t, k] = s // TILE
                    c_hi[t, k] = (s + n16[t, k] - 1) // TILE
                    inst_base[t, k] = n_inst
                    n_inst += c_hi[t, k] - c_lo[t, k] + 1
                else:
                    c_lo[t, k], c_hi[t, k] = 0, -1
                    inst_base[t, k] = n_inst
                s += n16[t, k]
            ns = -(-s // TILE) * TILE          # round call slots to 128
            call_slots[st, k] = ns
            st_cols += ns // TILE
        ncols_st[st] = st_cols

    idx_call_base = np.zeros((NST, 2), dtype=np.int64)
    b = [0, 0]
    for st in range(NST):
        for k in range(2):
            idx_call_base[st, k] = b[k]
            b[k] += call_slots[st, k]

    # per-edge positions
    t_g = tile_in_core
    e_slot = slot_base[t_g, chunk] + rank              # slot within call
    e_col_rel = e_slot // TILE                          # call-relative column
    e_part = e_slot % TILE
    e_inst = inst_base[t_g, chunk] + (e_col_rel - c_lo[t_g, chunk])

    per_core = []
    for c in range(M):
        m = core == c
        sc = src[m]
        ch, tg = chunk[m], t_g[m]

        flat_idx = [np.zeros(b[k], dtype=np.int16) for k in range(2)]
        for k in range(2):
            mk = ch == k
            pos = idx_call_base[tg[mk] // ST, k] + e_slot[m][mk]
            flat_idx[k][pos] = (sc[mk] - k * CHUNK).astype(np.int16)

        def wrap(flat, k):
            outs = []
            for st in range(NST):
                a = int(idx_call_base[st, k])
                n = int(call_slots[st, k])
                if n == 0:
                    continue
                blk = flat[a:a + n].reshape(n // 16, 16).T
                outs.append(np.tile(blk, (8, 1)))
            if not outs:
                return np.zeros((128, 1), np.int16)
            return np.ascontiguousarray(np.concatenate(outs, axis=1))

        idx0 = wrap(flat_idx[0], 0)
        idx1 = wrap(flat_idx[1], 1)

        dstl = np.full((TILE, max(n_inst, 1)), SENT, dtype=NP_BF16)
        dstl[e_part[m], e_inst[m]] = (row[m] - tg * TILE).astype(NP_BF16)

        per_core.append((idx0, idx1, np.ascontiguousarray(dstl)))

    layout = dict(nodes_by_core=nodes_by_core,
                  n16=n16, slot_base=slot_base, call_slots=call_slots,
                  call_col_base=call_col_base, ncols_st=ncols_st,
                  idx_call_base=idx_call_base, c_lo=c_lo, c_hi=c_hi,
                  inst_base=inst_base, n_inst=n_inst)
    return per_core, layout


def _build(layout, i0_cols, i1_cols, iters=1, mode="full"):
    call_slots = layout["call_slots"]
    call_col_base = layout["call_col_base"]
    ncols_st = layout["ncols_st"]
    idx_call_base = layout["idx_call_base"]
    c_lo, c_hi = layout["c_lo"], layout["c_hi"]
    inst_base = layout["inst_base"]
    ncols_total = int(max(layout["n_inst"], 1))
    max_ncols_st = int(ncols_st.max())
    max_ncols_tk = max(
        int(c_hi[t, k] - c_lo[t, k] + 1)
        for t in range(TPC) for k in range(2))

    nq = 4  # SWDGE queues (ucode max); gathers rotate across them so
    # descriptor generation pipelines with ring draining
    nc = bacc.Bacc(None, target_bir_lowering=False,
                   dynamic_dma_scratch_size=SCRATCH,
                   num_swdge_queues=nq)
    qctr = [0]
    h_pad = nc.dram_tensor("h_pad", [NPAD, GW], BF16, kind="ExternalInput")
    # h rows for each core-local dst row, pre-transposed on host:
    # h_self_t[p, t*128 + d] = h[node(t*128 + p), d]
    h_self_t = nc.dram_tensor("h_self_t", [TILE, TPC * TILE], BF16,
                              kind="ExternalInput")
    iota_in = nc.dram_tensor("iota", [TILE, TILE], BF16, kind="ExternalInput")
    iotar_in = nc.dram_tensor("iotar", [TILE, TILE * SCAP], BF16,
                              kind="ExternalInput")
    selfsel_in = nc.dram_tensor("selfsel", [TILE, TILE], BF16,
                                kind="ExternalInput")
    idx0_in = nc.dram_tensor("idx0", [128, i0_cols], mybir.dt.int16,
                             kind="ExternalInput")
    idx1_in = nc.dram_tensor("idx1", [128, i1_cols], mybir.dt.int16,
                             kind="ExternalInput")
    dstl_in = nc.dram_tensor("dstl", [TILE, ncols_total], BF16,
                             kind="ExternalInput")
    # transposed output: out_T[p, t*128 + d] = out_row(t*128 + p, d);
    # per-supertile DMA writes 1792B/partition descriptors instead of 256B
    out = nc.dram_tensor("out", [TILE, TPC * TILE], BF16,
                         kind="ExternalOutput")

    with TileContext(nc) as tc:
        with (
            tc.tile_pool(name="const", bufs=1) as cpool,
            tc.tile_pool(name="gbuf", bufs=4) as gpool,
            tc.tile_pool(name="idx", bufs=4) as ipool,
            tc.tile_pool(name="dstl", bufs=3) as dpool,
            tc.tile_pool(name="sel", bufs=2) as spool,
            tc.tile_pool(name="io", bufs=2) as iopool,
            tc.tile_pool(name="psum", bufs=4, space="PSUM") as ppool,
        ):
            iota_t = cpool.tile([TILE, TILE], BF16, tag="iota")
            nc.sync.dma_start(out=iota_t[:], in_=iota_in[:, :])
            iotar_t = cpool.tile([TILE, TILE * SCAP], BF16, tag="iotar")
            nc.sync.dma_start(out=iotar_t[:], in_=iotar_in[:, :])
            selfsel_t = cpool.tile([TILE, TILE], BF16, tag="selfsel")
            nc.sync.dma_start(out=selfsel_t[:], in_=selfsel_in[:, :])
            hs_all = cpool.tile([TILE, TPC * TILE], BF16, tag="hs")
            nc.sync.dma_start(out=hs_all[:], in_=h_self_t[:, :])

            if iters > 1:
                loop_cm = tc.For_i(
                    0, iters, 1,
                    hint_engines=(mybir.EngineType.Pool,
                                  mybir.EngineType.PE,
                                  mybir.EngineType.DVE,
                                  mybir.EngineType.SP,
                                  mybir.EngineType.Activation))
                loop_cm.__enter__()

            inst_col0 = 0
            for st in range(NST):
                n_inst_st = sum(
                    int(c_hi[t, k] - c_lo[t, k] + 1)
                    for t in range(st * ST, st * ST + ST) for k in range(2)
                    if c_hi[t, k] >= c_lo[t, k])
                gbuf = gpool.tile([TILE, max_ncols_st * GW], BF16, tag="gbuf")
                dstl_t = dpool.tile([TILE, max_ncols_st + ST * 2], BF16,
                                    tag="dstl")
                nc.sync.dma_start(
                    out=dstl_t[:, :n_inst_st],
                    in_=dstl_in[:, inst_col0:inst_col0 + n_inst_st],
                )
                for k, idx_in in ((0, idx0_in), (1, idx1_in)):
                    nslots = int(call_slots[st, k])
                    if nslots == 0:
                        continue
                    icols = nslots // 16
                    ibase = int(idx_call_base[st, k]) // 16
                    idx_t = ipool.tile([128, icols], mybir.dt.int16,
                                       tag=f"idx{k}")
                    nc.sync.dma_start(out=idx_t[:],
                                      in_=idx_in[:, ibase:ibase + icols])
                    a = int(call_col_base[st, k])
                    src_ap = h_pad[:CHUNK, :] if k == 0 else h_pad[CHUNK:, :]
                    # dma_gather is limited to <=GMAX indices per call
                    # (SWDGE descriptor carveout limit).
                    for p0 in range(0, nslots, GMAXE):
                        ps = min(GMAXE, nslots - p0)
                        pcols = -(-ps // TILE)
                        ac = a + p0 // TILE
                        gview = gbuf[:, ac * GW:(ac + pcols) * GW].rearrange(
                            "p (c d) -> p c d", d=GW)
                        nc.gpsimd.dma_gather(
                            gview, src_ap,
                            idx_t[:, p0 // 16:(p0 + ps) // 16],
                            ps, ps, GW, queue_num=qctr[0] % nq,
                            single_packet=SP)
                        qctr[0] += 1

                osb_all = iopool.tile([TILE, ST * TILE], BF16, tag="osb")
                for t in range(st * ST, st * ST + ST):
                    psum = ppool.tile([TILE, TILE], mybir.dt.float32, tag="ps")
                    first = True
                    for k in range(2):
                        if c_hi[t, k] < c_lo[t, k]:
                            continue
                        ncols_tk = int(c_hi[t, k] - c_lo[t, k] + 1)
                        ji0 = int(inst_base[t, k]) - inst_col0
                        for g0 in range(0, ncols_tk, SCAP):
                            gn = min(SCAP, ncols_tk - g0)
                            S = spool.tile([TILE, SCAP * TILE], BF16,
                                           tag="S")
                            # d-major S: S[p, d*gn + c] = (dstl[p, c] == d);
                            # all operands 2-byte packed-last for DVE 2x mode
                            nc.vector.tensor_tensor(
                                out=S[:, :gn * TILE].rearrange(
                                    "p (d c) -> p d c", c=gn),
                                in0=dstl_t[:, ji0 + g0:ji0 + g0 + gn
                                           ].rearrange(
                                    "p (o c) -> p o c", o=1).to_broadcast(
                                    [TILE, TILE, gn]),
                                in1=iotar_t[:, :].rearrange(
                                    "p (d c) -> p d c", c=SCAP)[:, :, :gn],
                                op=mybir.AluOpType.is_equal,
                            )
                            S_r = S[:, :gn * TILE].rearrange(
                                "p (d c) -> p c d", c=gn)
                            for ci in range(gn):
                                jr = int(call_col_base[st, k] + c_lo[t, k]
                                         ) + g0 + ci
                                nc.tensor.matmul(
                                    out=psum[:],
                                    lhsT=S_r[:, ci:ci + 1, :],
                                    rhs=gbuf[:, jr * GW:jr * GW + D],
                                    start=first,
                                    stop=False,
                                )
                                first = False
                    nc.tensor.matmul(
                        out=psum[:], lhsT=selfsel_t[:],
                        rhs=hs_all[:, t * TILE:(t + 1) * TILE],
                        start=first, stop=True)
                    i_t = t - st * ST
                    nc.scalar.activation(
                        osb_all[:, i_t * TILE:(i_t + 1) * TILE], psum[:],
                        mybir.ActivationFunctionType.Copy,
                        scale=1.0 - ALPHA)
                nc.sync.dma_start(
                    out=out[:, st * ST * TILE:(st + 1) * ST * TILE],
                    in_=osb_all[:])
                inst_col0 += n_inst_st
            if iters > 1:
                loop_cm.__exit__(None, None, None)
    nc.compile()
    return nc


def build_and_inputs(h, src, dst):
    """Returns (nc, in_maps) for the 8-core SPMD kernel."""
    h = np.ascontiguousarray(np.asarray(h, dtype=np.float32))
    src = np.asarray(src).astype(np.int64)
    dst = np.asarray(dst).astype(np.int64)

    per_core, layout = _prep(src, dst)
    h_bf = h.astype(NP_BF16)
    h_pad = np.zeros((NPAD, GW), NP_BF16)
    h_pad[:N, :D] = h_bf
    iota = np.broadcast_to(np.arange(TILE, dtype=np.float32), (TILE, TILE))
    iota = np.ascontiguousarray(iota.astype(NP_BF16))
    iotar = np.repeat(np.arange(TILE, dtype=np.float32), SCAP)
    iotar = np.broadcast_to(iotar, (TILE, TILE * SCAP))
    iotar = np.ascontiguousarray(iotar.astype(NP_BF16))
    selfsel = np.ascontiguousarray(
        (np.eye(TILE, dtype=np.float32) * (ALPHA / (1.0 - ALPHA))
         ).astype(NP_BF16))

    i0_cols = max(pc[0].shape[1] for pc in per_core)
    i1_cols = max(pc[1].shape[1] for pc in per_core)
    nc = _build(layout, i0_cols, i1_cols)

    in_maps = []
    for c in range(M):
        idx0, idx1, dstl = per_core[c]
        ids, rows = layout["nodes_by_core"][c]
        h_self = np.zeros((TPC * TILE, D), NP_BF16)
        h_self[rows] = h_bf[ids]
        h_self_t = np.ascontiguousarray(
            h_self.reshape(TPC, TILE, D).transpose(1, 0, 2).reshape(
                TILE, TPC * D))
        in_maps.append({
            "h_pad": h_pad,
            "h_self_t": h_self_t,
            "iota": iota,
            "iotar": iotar,
            "selfsel": selfsel,
            "idx0": idx0,
            "idx1": idx1,
            "dstl": dstl,
        })
    return nc, in_maps, layout


def kernel(h, src, dst, **_):
    global LAST_RESULT
    import os
    # NTFF tracing needs an axon hook that is absent in this environment;
    # make sure a stray BASS_TRACE can't break execution.
    os.environ["BASS_NEVER_TRACE"] = "1"
    nc, in_maps, layout = build_and_inputs(h, src, dst)
    res = run_bass_kernel_spmd(nc, in_maps, core_ids=list(range(M)))
    LAST_RESULT = res
    out = np.empty((N, D), np.float32)
    for c in range(M):
        ids, rows = layout["nodes_by_core"][c]
        arr = res.results[c]["out"]
        full = arr.reshape(TILE, TPC, TILE).transpose(1, 0, 2).reshape(
            TPC * TILE, TILE)
        out[ids] = full[rows].astype(np.float32)
    return out
